# revision 36
# baseline (speedup 1.0000x reference)
"""Causal self-attention with rotary embeddings (B=2, T=2048, D=1024, H=16,
d_k=64) on 8 Trainium2 NeuronCores.

Sharding: core c handles batch b = c//4 and 4 heads (c%4)*4..+4 — data
parallel on B, tensor parallel on heads.  Each core computes its heads'
qkv projection, RoPE, causal attention, and a partial output projection
over its 256 attention channels; the host sums the 4 partials per batch.

Layout/perf notes:
  * everything is bf16 except PSUM accumulation, softmax scores (f32 in
    PSUM) and the reciprocal: x/Wqkv/Wout are cast host-side, q/k post
    rope, exp outputs, v, attn and the output partials are bf16
    (validated ~4e-3 rel err vs the 2e-2 gate).
  * q/k channels are de-interleaved host-side (RoPE pair -> half-split
    form) and packed 2 heads per 128-partition tile; the two heads'
    64-contraction score matmuls co-execute on separate PE row tiles.
  * RoPE swap (+/- sign) is a 128x128 permutation matmul on TensorE;
    cos/sin tables are natural scale, the 1/sqrt(d_k) folds into the
    exp's scale operand on the activation engine.
  * causal structure at 128-column granularity: score/av matmuls and
    exp only cover columns >= the key chunk's diagonal.  The intra-block
    mask of the diagonal 128x128 is applied on TensorE by accumulating
    -800 * triu(,1) into the scores PSUM (exp(0.125*(s-800)) == 0 in
    bf16), keeping the DVE out of the score->exp->av chain.
  * softmax denominator rides as a ones-column of v through the av
    matmul; reciprocal reads the PSUM row directly, gpsimd broadcasts,
    vector applies it during attnT eviction.
  * engine budget per core (warm): PE ~115us, ScalarE ~84us (exp is
    1 elem/lane/cycle @1.2GHz and irreducible), DVE ~70us.  The exp
    deficit inside attention waves is filled with qk/v projection
    chunks (waves 0-2) and all of the output projection (wave 3).
  * zero bias (the spec fills bqkv with zeros) skips the bias rank-1
    matmuls entirely; a with_bias program variant keeps generality.
"""

import sys

sys.path.insert(0, "/opt/trn_rl_repo")

import numpy as np
import ml_dtypes

import concourse.bacc as bacc
import concourse.tile as tile
from concourse import mybir
from concourse.bass_utils import run_bass_kernel_spmd

F32 = mybir.dt.float32
F32R = mybir.dt.float32r
BF16 = mybir.dt.bfloat16

B, T, D = 2, 2048, 1024
NH, DK = 16, 64
THETA = 10000.0
NCORES = 8
HEADS_PER_CORE = 4

TC512 = T // 512        # 4   i-chunks of 512
TC128 = T // 128        # 16  t/j-chunks of 128
KC = D // 128           # 8   d_model contraction chunks

MASK_BIAS = -800.0      # pre-scale; exp(0.125 * -800) flushes to 0

EXP = mybir.ActivationFunctionType.Exp
COPYF = mybir.ActivationFunctionType.Copy


def build_program(with_bias=False, debug=False):
    nc = bacc.Bacc("TRN2", target_bir_lowering=False, debug=False)

    XT = nc.dram_tensor("XT", [D + 1, T], BF16, kind="ExternalInput").ap()
    WQK = nc.dram_tensor("WQK", [D + 1, 512], BF16, kind="ExternalInput").ap()
    WV = nc.dram_tensor("WV", [D + 1, 256], BF16, kind="ExternalInput").ap()
    WOUT = nc.dram_tensor("WOUT", [256, D], BF16, kind="ExternalInput").ap()
    PT = nc.dram_tensor("PT", [128, 384], BF16, kind="ExternalInput").ap()
    CSQ = nc.dram_tensor("CSQ", [128, 2 * T], BF16, kind="ExternalInput").ap()
    ONES64 = nc.dram_tensor("ONES64", [1, 64], F32R, kind="ExternalInput").ap()
    OUT = nc.dram_tensor("OUT", [T, D], BF16, kind="ExternalOutput").ap()
    if debug:
        DBG_QKT = nc.dram_tensor("DBG_QKT", [128, 4 * T], BF16, kind="ExternalOutput").ap()
        DBG_V = nc.dram_tensor("DBG_V", [128, TC128 * 260], BF16, kind="ExternalOutput").ap()
        DBG_ATT = nc.dram_tensor("DBG_ATT", [128, 2 * T], BF16, kind="ExternalOutput").ap()

    with tile.TileContext(nc) as tc:
        with (
            tc.tile_pool(name="persist", bufs=1) as persist,
            tc.tile_pool(name="p1w", bufs=1) as p1w,
            tc.tile_pool(name="p1t", bufs=3) as p1t,
            tc.tile_pool(name="p2e", bufs=6) as p2e,
            tc.tile_pool(name="p2bc", bufs=2) as p2bc,
            tc.tile_pool(name="p2r", bufs=2) as p2r,
            tc.tile_pool(name="pj", bufs=2, space="PSUM") as pj,
            tc.tile_pool(name="sps", bufs=2, space="PSUM") as sps,
            tc.tile_pool(name="avps", bufs=2, space="PSUM") as avps,
        ):
            # ---- persistent tiles --------------------------------------
            qkT = persist.tile([128, 4 * T], BF16, tag="qkT")       # Qp0 Kp0 Qp1 Kp1
            v_sb = persist.tile([128, TC128 * 260], BF16, tag="v_sb")  # [jc, head, 64+1]
            attnT = persist.tile([128, 2 * T], BF16, tag="attnT")   # c-chunks x t
            wout_sb = persist.tile([128, 2 * D], BF16, tag="wout_sb")
            ones_sb = persist.tile([1, 64], F32R, tag="ones_sb")
            warm_sb = persist.tile([1, 8], F32, tag="warm_sb")

            x_sb = p1w.tile([128, KC * T], BF16, tag="x_sb")
            wqk_sb = p1w.tile([128, KC * 512], BF16, tag="wqk_sb")
            wv_sb = p1w.tile([128, KC * 256], BF16, tag="wv_sb")
            pt_sb = p1w.tile([128, 384], BF16, tag="pt_sb")
            psw_sb = pt_sb[:, 0:128]
            triu_sb = pt_sb[:, 128:256]
            negi_sb = pt_sb[:, 256:384]
            csq_sb = p1w.tile([128, 2 * T], BF16, tag="csq_sb")
            cq_sb = csq_sb[:, 0:T]
            sq_sb = csq_sb[:, T:2 * T]
            if with_bias:
                xlast = p1w.tile([1, T], BF16, tag="xlast")
                wqk_last = p1w.tile([1, 512], BF16, tag="wqk_last")
                wv_last = p1w.tile([1, 256], BF16, tag="wv_last")

            xt_src = XT[0:D, :].rearrange("(k p) t -> p k t", p=128)
            x_dst = x_sb[:].rearrange("p (k t) -> p k t", k=KC)

            # x block n: 2 batched triggers.  Strided multi-k transfers
            # cost ~10us of descriptor generation on the HWDGE engines
            # (sync/scalar) but ~1us on gpsimd's SWDGE — always gpsimd.
            def load_x_block(n, engines=(nc.gpsimd, nc.gpsimd)):
                nsl = slice(n * 512, (n + 1) * 512)
                for half, eng in enumerate(engines):
                    ks = slice(half * 4, half * 4 + 4)
                    eng.dma_start(x_dst[:, ks, nsl], xt_src[:, ks, nsl])

            # ---- preamble loads ----------------------------------------
            # k-chunk granular, pipelined to match the PE's consumption
            # order (one (wqk_k, x_k) pair per ~0.85us), spread across 4
            # DMA-trigger queues (sync/gpsimd/scalar/vector) so triggers
            # don't serialize on one engine.
            wqk_dst = wqk_sb[:].rearrange("p (k c) -> p k c", k=KC)
            wqk_src = WQK[0:D, :].rearrange("(k p) c -> p k c", p=128)
            wv_dst = wv_sb[:].rearrange("p (k c) -> p k c", k=KC)
            wv_src = WV[0:D, :].rearrange("(k p) c -> p k c", p=128)

            # warm the exp table on ScalarE while DMA ramps (2.7us load)
            nc.vector.memset(warm_sb[:], 0.0)
            nc.scalar.activation(warm_sb[:], warm_sb[:], EXP, scale=0.125)

            # Preamble loads use few BIG triggers: per-trigger latency
            # (~2us descriptor-gen + doorbell) caps a queue at ~65GB/s on
            # 131KB chunks, so half-tensor (0.5MB) transfers are what let
            # the critical 2.1MB land in <10us.  Two queues carry
            # complementary halves in consumption order; pt/csq/wv ride
            # the third.
            # Queue FIFOs transfer in order, but the trigger engines race
            # ahead and enqueue everything immediately — so the k0
            # criticals must be the FIRST trigger on their queue, with all
            # later-needed blocks queued BEHIND them.  k0 rides alone
            # (small first transfer => earliest first matmul); the sync
            # queue observes ~5us extra first-packet latency, so it gets
            # the later-needed halves.
            # Three DMA rings share the 16 SDMA engines round-robin at
            # packet granularity, and each ring is a strict FIFO — so the
            # rings carry BALANCED, consumption-ordered critical loads
            # (sync: wqk; scalar: x@n0; gpsimd: pt+csq), with everything
            # later-needed strictly behind them on the same rings.
            nc.sync.dma_start(wqk_dst[:, 0:1], wqk_src[:, 0:1])
            nc.scalar.dma_start(x_dst[:, 0:1, 0:512], xt_src[:, 0:1, 0:512])
            nc.gpsimd.dma_start(pt_sb[:], PT[:])
            nc.sync.dma_start(wqk_dst[:, 1:4], wqk_src[:, 1:4])
            nc.scalar.dma_start(x_dst[:, 1:4, 0:512], xt_src[:, 1:4, 0:512])
            nc.gpsimd.dma_start(csq_sb[:], CSQ[:])
            nc.sync.dma_start(wqk_dst[:, 4:8], wqk_src[:, 4:8])
            nc.scalar.dma_start(x_dst[:, 4:8, 0:512], xt_src[:, 4:8, 0:512])
            nc.gpsimd.dma_start(wv_dst[:, 0:8], wv_src[:, 0:8])
            nc.gpsimd.dma_start(ones_sb[:], ONES64[:])
            if with_bias:
                nc.gpsimd.dma_start(wqk_last[:], WQK[D:D + 1, :])
                nc.gpsimd.dma_start(xlast[:], XT[D:D + 1, :])
                nc.gpsimd.dma_start(wv_last[:], WV[D:D + 1, :])
            # x block n=1 early (consumed by qk_proj(.,1) inside wave 0)
            load_x_block(1)

            # ones columns of v_aug: one strided memset
            v4 = v_sb[:].rearrange("p (jc h e) -> p jc h e", jc=TC128, h=4)
            nc.vector.memset(v4[:, :, :, 64:65], 1.0)

            # ---------------- building blocks ---------------------------
            def qk_proj_chunk(m, n):
                """project q/k m-chunk (128 channels) for t-chunk n (512), apply rope."""
                nsl = slice(n * 512, (n + 1) * 512)
                ps = pj.tile([128, 512], F32, tag="pj", name=f"psqk_{m}_{n}")
                for k in range(KC):
                    nc.tensor.matmul(
                        ps[:],
                        wqk_sb[:, k * 512 + m * 128:k * 512 + (m + 1) * 128],
                        x_sb[:, k * T + n * 512:k * T + (n + 1) * 512],
                        start=(k == 0), stop=(not with_bias and k == KC - 1),
                    )
                if with_bias:
                    nc.tensor.matmul(
                        ps[:], wqk_last[:, m * 128:(m + 1) * 128], xlast[:, nsl],
                        start=False, stop=True,
                    )
                tmp_s = p1t.tile([128, 512], BF16, tag="tmp_s", name=f"tmps_{m}_{n}")
                tmp_c = p1t.tile([128, 512], BF16, tag="tmp_c", name=f"tmpc_{m}_{n}")
                nc.vector.tensor_mul(tmp_s[:], ps[:], sq_sb[:, nsl])
                nc.vector.tensor_mul(tmp_c[:], ps[:], cq_sb[:, nsl])
                sw = pj.tile([128, 512], F32, tag="pj", name=f"sw_{m}_{n}")
                nc.tensor.matmul(sw[:], psw_sb, tmp_s[:], start=True, stop=True)
                nc.vector.tensor_add(qkT[:, m * T + n * 512:m * T + (n + 1) * 512], sw[:], tmp_c[:])

            def qk_proj_halves(m, n):
                """qk_proj_chunk split into two filler callables (finer PE
                filler granularity => smaller exp bubbles).  The halves
                share the ps accumulation tile; they must stay adjacent in
                the filler sequence (nothing else may allocate from pj in
                between)."""
                nsl = slice(n * 512, (n + 1) * 512)
                st = {}

                def half_a():
                    ps = st["ps"] = pj.tile([128, 512], F32, tag="pj", name=f"psqk_{m}_{n}")
                    for k in range(4):
                        nc.tensor.matmul(
                            ps[:],
                            wqk_sb[:, k * 512 + m * 128:k * 512 + (m + 1) * 128],
                            x_sb[:, k * T + n * 512:k * T + (n + 1) * 512],
                            start=(k == 0), stop=False, skip_group_check=True,
                        )

                def half_b():
                    ps = st["ps"]
                    for k in range(4, KC):
                        nc.tensor.matmul(
                            ps[:],
                            wqk_sb[:, k * 512 + m * 128:k * 512 + (m + 1) * 128],
                            x_sb[:, k * T + n * 512:k * T + (n + 1) * 512],
                            start=False, stop=(not with_bias and k == KC - 1),
                            skip_group_check=True,
                        )
                    if with_bias:
                        nc.tensor.matmul(
                            ps[:], wqk_last[:, m * 128:(m + 1) * 128], xlast[:, nsl],
                            start=False, stop=True, skip_group_check=True,
                        )
                    tmp_s = p1t.tile([128, 512], BF16, tag="tmp_s", name=f"tmps_{m}_{n}")
                    tmp_c = p1t.tile([128, 512], BF16, tag="tmp_c", name=f"tmpc_{m}_{n}")
                    nc.vector.tensor_mul(tmp_s[:], ps[:], sq_sb[:, nsl])
                    nc.vector.tensor_mul(tmp_c[:], ps[:], cq_sb[:, nsl])
                    sw = pj.tile([128, 512], F32, tag="pj", name=f"sw_{m}_{n}")
                    nc.tensor.matmul(sw[:], psw_sb, tmp_s[:], start=True, stop=True)
                    nc.vector.tensor_add(qkT[:, m * T + n * 512:m * T + (n + 1) * 512], sw[:], tmp_c[:])

                return [half_a, half_b]

            def v_proj_chunk(tcc):
                tsl = slice(tcc * 128, (tcc + 1) * 128)
                psv = pj.tile([128, 256], F32, tag="pj", name=f"psv_{tcc}")
                for k in range(KC):
                    nc.tensor.matmul(
                        psv[:],
                        x_sb[:, k * T + tcc * 128:k * T + (tcc + 1) * 128],
                        wv_sb[:, k * 256:(k + 1) * 256],
                        start=(k == 0), stop=(not with_bias and k == KC - 1),
                    )
                if with_bias:
                    nc.tensor.matmul(psv[:], xlast[:, tsl], wv_last[:], start=False, stop=True)
                vdst = v_sb[:, tcc * 260:(tcc + 1) * 260].rearrange(
                    "p (h e) -> p h e", h=4)[:, :, 0:64]
                vsrc = psv[:].rearrange("p (h e) -> p h e", e=64)
                nc.vector.tensor_copy(vdst, vsrc)

            # exp'd score tiles, keyed (p, ic, jc) — shared between the
            # cross-block prescore fillers and the owning attn_ic
            e_shared = {}

            def score_unit(p, ic, jc):
                """scores+mask+exp for one (p, ic, jc) block; leaves the
                exp'd bf16 tile in e_shared for the AV matmuls."""
                qof = (2 * p) * T
                kof = (2 * p + 1) * T
                rel = jc - 4 * ic
                ls = 0 if rel < 0 else rel * 128
                e_pair = p2e.tile([128, 1024], BF16, tag="e_t", name=f"e_{p}_{ic}_{jc}")
                s_pair = sps.tile([128, 1024], F32, tag="s_ps", name=f"s_{p}_{ic}_{jc}")
                for hh in range(2):
                    nc.tensor.matmul(
                        s_pair[:, hh * 512 + ls:(hh + 1) * 512],
                        qkT[hh * 64:hh * 64 + 64, kof + jc * 128:kof + (jc + 1) * 128],
                        qkT[hh * 64:hh * 64 + 64, qof + ic * 512 + ls:qof + (ic + 1) * 512],
                        start=True, stop=True,
                    )
                if rel >= 0:
                    # intra-block causal mask: accumulate -800*triu(,1)
                    # into the diagonal 128x128 (TensorE, pre-exp)
                    for hh in range(2):
                        nc.tensor.matmul(
                            s_pair[:, hh * 512 + ls:hh * 512 + ls + 128],
                            triu_sb, negi_sb,
                            start=False, stop=True,
                            skip_group_check=True,
                        )
                sv = s_pair[:].rearrange("p (h w) -> p h w", h=2)
                ev = e_pair[:].rearrange("p (h w) -> p h w", h=2)
                nc.scalar.activation(ev[:, :, ls:512], sv[:, :, ls:512], EXP, scale=0.125)
                e_shared[(p, ic, jc)] = e_pair

            def attn_ic(p, ic, fillers=(), mid=None, skip_scores=0):
                """attention for head-pair p, query chunk ic (512 queries).
                fillers: callables run one per jc iteration (PE density).
                mid: when set (wave 0 only, njc <= e-pool bufs), all
                scores+exp are issued first, mid() runs, then the AV
                matmuls.  skip_scores: this many leading jc blocks were
                already issued by prescore fillers of the previous block
                (keeps ScalarE busy across block boundaries)."""
                fillers = list(fillers)
                njc = 4 * ic + 4
                av = [avps.tile([65, 512], F32, tag="av", name=f"av_{p}_{ic}_{i}") for i in range(2)]

                def av_jc(jc):
                    rel = jc - 4 * ic
                    ls = 0 if rel < 0 else rel * 128
                    e_pair = e_shared.pop((p, ic, jc))
                    for hh in range(2):
                        nc.tensor.matmul(
                            av[hh][:, ls:512],
                            v_sb[:, jc * 260 + (2 * p + hh) * 65:jc * 260 + (2 * p + hh) * 65 + 65],
                            e_pair[:, hh * 512 + ls:(hh + 1) * 512],
                            start=(jc == 0), stop=(jc == njc - 1),
                            skip_group_check=True,
                        )

                if mid is not None:
                    assert njc <= 4 and skip_scores == 0
                    for jc in range(njc):
                        score_unit(p, ic, jc)
                    mid()
                    for jc in range(njc):
                        av_jc(jc)
                        if fillers and (jc % max(1, njc // len(fillers)) == 0 or jc == njc - 1):
                            while fillers and len(fillers) > (njc - 1 - jc):
                                fillers.pop(0)()
                else:
                    # software-pipelined one jc ahead: scores(jc+1) issues
                    # before av(jc), so av's wait on exp(jc) is covered by
                    # independent PE work instead of an exposed stall.
                    # The first AV also waits for the PREVIOUS block's
                    # PSUM av tiles to be released by its normalization
                    # chain (~3us) — run a few fillers first to cover it.
                    issued = skip_scores
                    if issued == 0:
                        score_unit(p, ic, 0)
                        issued = 1
                    for _ in range(min(len(fillers), 3)):
                        fillers.pop(0)()
                    for jc in range(njc):
                        if issued < min(njc, jc + 2):
                            score_unit(p, ic, issued)
                            issued += 1
                        av_jc(jc)
                        if fillers and (jc % max(1, njc // len(fillers)) == 0 or jc == njc - 1):
                            while fillers and len(fillers) > (njc - 1 - jc):
                                fillers.pop(0)()
                # normalization, phase-interleaved so the gpsimd broadcasts
                # overlap the vector ops instead of serializing per head
                dens, recs, bcs = [], [], []
                for hh in range(2):
                    den = p2r.tile([1, 512], F32, tag="den", name=f"den_{p}_{ic}_{hh}")
                    nc.vector.tensor_copy(den[:], av[hh][64:65, :])
                    dens.append(den)
                for hh in range(2):
                    rec = p2r.tile([1, 512], F32, tag="rec", name=f"rec_{p}_{ic}_{hh}")
                    nc.vector.reciprocal_approx_fast(rec[:], dens[hh][:])
                    recs.append(rec)
                    bc_sb = p2bc.tile([64, 512], F32, tag="bc_sb", name=f"bc_{p}_{ic}_{hh}")
                    nc.gpsimd.partition_broadcast(bc_sb[:], rec[:], channels=64)
                    bcs.append(bc_sb)
                for hh in range(2):
                    head = 2 * p + hh
                    cof = (head // 2) * T
                    pof = (head % 2) * 64
                    dst = attnT[pof:pof + 64, cof + ic * 512:cof + (ic + 1) * 512]
                    nc.vector.tensor_mul(dst, av[hh][0:64, :], bcs[hh][:])

            def out_proj_chunk(tcc, dma_eng=None, split=False):
                """output projection for token chunk tcc.  split=True (tail):
                per-oc eviction on alternating engines + per-oc DMA on two
                queues so the last chunks drain with minimal serial chain."""
                tsl = slice(tcc * 128, (tcc + 1) * 128)
                po_sb = p1t.tile([128, 1024], BF16, tag="po_sb", name=f"po_sb_{tcc}")
                for oc in range(2):
                    po = pj.tile([128, 512], F32, tag="pj", name=f"po_{tcc}_{oc}")
                    for cc in range(2):
                        nc.tensor.matmul(
                            po[:],
                            attnT[:, cc * T + tcc * 128:cc * T + (tcc + 1) * 128],
                            wout_sb[:, cc * D + oc * 512:cc * D + (oc + 1) * 512],
                            start=(cc == 0), stop=(cc == 1),
                        )
                    osl = slice(oc * 512, (oc + 1) * 512)
                    if split:
                        ev = (nc.vector.tensor_copy, nc.scalar.copy)[oc]
                        ev(po_sb[:, osl], po[:])
                        (nc.sync, nc.gpsimd)[oc].dma_start(OUT[tsl, osl], po_sb[:, osl])
                    else:
                        nc.vector.tensor_copy(po_sb[:, osl], po[:])
                if not split:
                    (dma_eng or nc.sync).dma_start(OUT[tsl, :], po_sb[:])

            # ---------------- schedule: static list scheduler -----------
            # One global interleaved stream, built by simulating coarse
            # PE/ACT/DVE clocks and DMA arrivals.  Goals: ScalarE (exp)
            # never starves once data exists, every PE dependency wait is
            # covered by independent matmuls already in the queue, and
            # the projections ride wherever the exp deficit needs them.
            def load_wout():
                nc.gpsimd.dma_start(wout_sb[:, 0:D], WOUT[0:128, :])
                nc.gpsimd.dma_start(wout_sb[:, D:2 * D], WOUT[128:256, :])

            # remaining loads: behind the criticals on the gpsimd ring,
            # in consumption order — issue the triggers now.
            load_x_block(2)
            load_x_block(3)
            load_wout()

            # modeled DMA arrival times (ns, calibrated from traces)
            ARR_XK = {k: 11000 if k == 0 else (15000 if k < 4 else 19000)
                      for k in range(KC)}
            ARR = {
                "csq": 15000, "wv": 21000, "wout": 40000,
                "xn": {0: 19000, 1: 26000, 2: 33000, 3: 39000},
            }

            BLOCKS = [(ic, p) for ic in range(TC512) for p in range(2)]
            njc_of = lambda ic: 4 * ic + 4

            # ---- unit tables ----
            qk_done = {}      # (m, n) -> modeled qkT-ready time
            v_done = {}       # tcc -> modeled v_sb-ready time
            exp_done = {}     # (p, ic, jc) -> modeled exp completion
            av_tiles = {}     # (ic, p) -> [av0, av1]
            norm_done = {}    # (ic, p) -> modeled av-tiles-released time

            clk = {"PE": 10000.0, "ACT": 10000.0, "DVE": 10000.0}

            def w_of(jc, ic):
                rel = jc - 4 * ic
                return 512 - (0 if rel < 0 else rel * 128), rel

            qk_halves = {}
            for n in range(TC512):
                for m in range(4):
                    qk_halves[(m, n)] = qk_proj_halves(m, n)

            def x_arr(n):
                return ARR_XK[7] if n == 0 else ARR["xn"][n]

            def emit_qk(m, n, half):
                qk_halves[(m, n)][half]()
                if half == 0:
                    clk["PE"] = max(clk["PE"], ARR_XK[3] if n == 0 else ARR["xn"][n]) + 960
                else:
                    t = max(clk["PE"], x_arr(n), ARR["csq"]) + 1200
                    clk["PE"] = t
                    clk["DVE"] = max(clk["DVE"], t) + 2100
                    qk_done[(m, n)] = clk["DVE"] + 300

            def emit_v(tcc):
                v_proj_chunk(tcc)
                t = max(clk["PE"], ARR["wv"], x_arr(tcc // 4)) + 1040
                clk["PE"] = t
                clk["DVE"] = max(clk["DVE"], t) + 500
                v_done[tcc] = clk["DVE"]

            def emit_sc(ic, p, jc):
                score_unit(p, ic, jc)
                W, rel = w_of(jc, ic)
                t = max(clk["PE"],
                        qk_done[(2 * p, ic)], qk_done[(2 * p + 1, jc // 4)])
                t += W / 2.4 + 45 + (110 if rel >= 0 else 0)
                clk["PE"] = t
                start = max(clk["ACT"], t)
                clk["ACT"] = start + (2 * W + 390) / 1.2
                exp_done[(p, ic, jc)] = clk["ACT"]

            def emit_av(ic, p, jc):
                key = (ic, p)
                if key not in av_tiles:
                    av_tiles[key] = [avps.tile([65, 512], F32, tag="av",
                                               name=f"av_{p}_{ic}_{i}") for i in range(2)]
                av = av_tiles[key]
                njc = njc_of(ic)
                W, rel = w_of(jc, ic)
                ls = 512 - W
                e_pair = e_shared.pop((p, ic, jc))
                for hh in range(2):
                    nc.tensor.matmul(
                        av[hh][:, ls:512],
                        v_sb[:, jc * 260 + (2 * p + hh) * 65:jc * 260 + (2 * p + hh) * 65 + 65],
                        e_pair[:, hh * 512 + ls:(hh + 1) * 512],
                        start=(jc == 0), stop=(jc == njc - 1),
                        skip_group_check=True,
                    )
                t = max(clk["PE"], exp_done[(p, ic, jc)], v_done[jc])
                if jc == 0:
                    bi = BLOCKS.index((ic, p))
                    if bi > 0:
                        t = max(t, norm_done.get(BLOCKS[bi - 1], 0))
                clk["PE"] = t + 2 * W / 2.4 + 90

            def emit_norm(ic, p):
                av = av_tiles[(ic, p)]
                dens, bcs = [], []
                for hh in range(2):
                    den = p2r.tile([1, 512], F32, tag="den", name=f"den_{p}_{ic}_{hh}")
                    nc.vector.tensor_copy(den[:], av[hh][64:65, :])
                    dens.append(den)
                for hh in range(2):
                    rec = p2r.tile([1, 512], F32, tag="rec", name=f"rec_{p}_{ic}_{hh}")
                    nc.vector.reciprocal_approx_fast(rec[:], dens[hh][:])
                    bc_sb = p2bc.tile([64, 512], F32, tag="bc_sb", name=f"bc_{p}_{ic}_{hh}")
                    nc.gpsimd.partition_broadcast(bc_sb[:], rec[:], channels=64)
                    bcs.append(bc_sb)
                for hh in range(2):
                    head = 2 * p + hh
                    cof = (head // 2) * T
                    pof = (head % 2) * 64
                    dst = attnT[pof:pof + 64, cof + ic * 512:cof + (ic + 1) * 512]
                    nc.vector.tensor_mul(dst, av[hh][0:64, :], bcs[hh][:])
                clk["DVE"] = max(clk["DVE"], clk["PE"]) + 2800
                norm_done[(ic, p)] = clk["DVE"] + 1300

            def emit_out(tcc, split=False):
                out_proj_chunk(tcc, (nc.sync, nc.gpsimd)[tcc % 2], split=split)
                clk["PE"] = max(clk["PE"], norm_done[(tcc // 4, 0)],
                                norm_done[(tcc // 4, 1)], ARR["wout"]) + 980
                clk["DVE"] = max(clk["DVE"], clk["PE"]) + 1360

            # ---- work lists ----
            sc_todo = [(ic, p, jc) for ic, p in BLOCKS for jc in range(njc_of(ic))]
            av_left = {(ic, p): 0 for ic, p in BLOCKS}
            qk_todo = [(m, n, h) for n in range(TC512) for m in range(4) for h in range(2)]
            v_todo = list(range(TC128))
            out_todo = list(range(TC128))
            sc_emitted = set()
            e_inflight = 0
            bi_next = 0               # index into BLOCKS whose AVs are active

            def sc_ready(u):
                ic, p, jc = u
                return ((2 * p, ic) in qk_done and (2 * p + 1, jc // 4) in qk_done
                        and e_inflight < 5)

            def do_sc():
                u = sc_todo.pop(0)
                emit_sc(*u)
                sc_emitted.add(u)

            def do_av():
                nonlocal bi_next, e_inflight
                ic, p = BLOCKS[bi_next]
                jc = av_left[(ic, p)]
                emit_av(ic, p, jc)
                e_inflight -= 1
                av_left[(ic, p)] += 1
                if av_left[(ic, p)] == njc_of(ic):
                    emit_norm(ic, p)
                    del av_left[(ic, p)]
                    bi_next += 1

            def av_ok(lenient):
                if bi_next >= len(BLOCKS):
                    return False
                ic, p = BLOCKS[bi_next]
                jc = av_left[(ic, p)]
                if (ic, p, jc) not in sc_emitted or jc not in v_done:
                    return False
                return lenient or exp_done[(p, ic, jc)] <= clk["PE"] + 300

            def out_ok(lenient):
                if not out_todo:
                    return False
                tcc = out_todo[0]
                if (tcc // 4, 0) not in norm_done or (tcc // 4, 1) not in norm_done:
                    return False
                return lenient or clk["PE"] + 1500 >= ARR["wout"]

            guard = 0
            while sc_todo or av_left or out_todo or qk_todo or v_todo:
                guard += 1
                assert guard < 5000, "scheduler wedged"
                sc_now = sc_todo and sc_ready(sc_todo[0])
                # 1. feed ScalarE whenever it would run dry soon
                if sc_now and clk["ACT"] < clk["PE"] + 2500:
                    e_inflight += 1
                    do_sc()
                    continue
                # 2. AVs whose exp is done (frees e-pool, advances blocks)
                if av_ok(False):
                    do_av()
                    continue
                # 3. fillers: v needed soon > qk > out, data-arrival gated
                jc_need = (av_left[BLOCKS[bi_next]]
                           if bi_next < len(BLOCKS) else 99)
                if v_todo and v_todo[0] <= jc_need + 2 and \
                        clk["PE"] + 2500 >= max(ARR["wv"], x_arr(v_todo[0] // 4)):
                    emit_v(v_todo.pop(0))
                    continue
                if qk_todo and clk["PE"] + 3500 >= x_arr(qk_todo[0][1]):
                    m, n, h = qk_todo.pop(0)
                    emit_qk(m, n, h)
                    continue
                if v_todo and clk["PE"] + 2500 >= \
                        max(ARR["wv"], x_arr(v_todo[0] // 4)):
                    emit_v(v_todo.pop(0))
                    continue
                if out_ok(False):
                    emit_out(out_todo.pop(0), split=(len(out_todo) <= 4 and not sc_todo))
                    continue
                # 4. nothing naturally ready: force progress
                if sc_now:
                    e_inflight += 1
                    do_sc()
                    continue
                if av_ok(True):
                    do_av()
                    continue
                if qk_todo:
                    m, n, h = qk_todo.pop(0)
                    emit_qk(m, n, h)
                    continue
                if v_todo:
                    emit_v(v_todo.pop(0))
                    continue
                if out_ok(True):
                    emit_out(out_todo.pop(0), split=(len(out_todo) <= 4))
                    continue
                raise RuntimeError("scheduler deadlock")

            if debug:
                nc.sync.dma_start(DBG_QKT[:], qkT[:])
                nc.sync.dma_start(DBG_V[:], v_sb[:])
                nc.sync.dma_start(DBG_ATT[:], attnT[:])

    nc.compile()
    return nc


_DEINT = list(range(0, DK, 2)) + list(range(1, DK, 2))


def _rope_tables():
    j = np.arange(DK // 2, dtype=np.float64)
    inv_freq = THETA ** (-2.0 * j / DK)
    t = np.arange(T, dtype=np.float64)
    ang = t[None, :] * inv_freq[:, None]          # [32, T]
    ang = np.tile(ang, (4, 1))                    # [128, T]
    return np.cos(ang), np.sin(ang)


def _psw():
    M = np.zeros((128, 128), dtype=np.float32)
    for p in range(128):
        pm = p % 64
        if pm < 32:
            M[p, p + 32] = -1.0
        else:
            M[p, p - 32] = 1.0
    return np.ascontiguousarray(M.T)


def shard_inputs(x, Wqkv, bqkv, Wout, bout):
    bf = ml_dtypes.bfloat16
    x = np.asarray(x, dtype=np.float32)
    Wqkv = np.asarray(Wqkv, dtype=np.float32)
    bqkv = np.asarray(bqkv, dtype=np.float32)
    Wout = np.asarray(Wout, dtype=np.float32)

    cos_t, sin_t = _rope_tables()
    csq = np.ascontiguousarray(
        np.concatenate([cos_t, sin_t], axis=1)).astype(bf)   # [128, 2T]
    psw = _psw()
    # stationary strict-upper mask (lhsT[c,p] = 1 for key p > query c) and
    # the -800*I moving operand for the diagonal-block mask matmul
    triu = np.triu(np.ones((128, 128), dtype=np.float32), 1)
    negi = MASK_BIAS * np.eye(128, dtype=np.float32)
    pt = np.ascontiguousarray(np.concatenate([psw, triu, negi], axis=1)).astype(bf)
    ones64 = np.ones((1, 64), dtype=np.float32)

    Wfull = np.concatenate([Wqkv, bqkv[:, None]], axis=1)  # [3072, 1025]

    xt = {}
    for b in range(B):
        xt[b] = np.ascontiguousarray(
            np.concatenate([x[b].T, np.ones((1, T), np.float32)], axis=0)
        ).astype(bf)

    in_maps = []
    for c in range(NCORES):
        b = c // 4
        heads = [4 * (c % 4) + i for i in range(HEADS_PER_CORE)]
        # chunk order: [Qp0 | Kp0 | Qp1 | Kp1], each 128 rows (2 heads x 64)
        qk_rows = []
        for p in range(2):
            qrows, krows = [], []
            for h in (2 * p, 2 * p + 1):
                H = heads[h]
                qrows += [H * 192 + j for j in _DEINT]
                krows += [H * 192 + 64 + j for j in _DEINT]
            qk_rows += qrows + krows
        v_rows = []
        for h in range(4):
            H = heads[h]
            v_rows += [H * 192 + 128 + j for j in range(DK)]
        vch_out = []
        for h in range(4):
            H = heads[h]
            vch_out += [H * 64 + j for j in range(DK)]

        in_maps.append({
            "XT": xt[b],
            "WQK": np.ascontiguousarray(Wfull[qk_rows].T).astype(bf),
            "WV": np.ascontiguousarray(Wfull[v_rows].T).astype(bf),
            "WOUT": np.ascontiguousarray(Wout[:, vch_out].T).astype(bf),
            "PT": pt,
            "CSQ": csq,
            "ONES64": ones64,
        })
    return in_maps


_CACHED = {}


def _get_program(with_bias=False, debug=False):
    key = (bool(with_bias), bool(debug))
    if key not in _CACHED:
        _CACHED[key] = build_program(with_bias=with_bias, debug=debug)
    return _CACHED[key]


def run_cores(inputs, debug=False, trace=False, tmpdir=None):
    with_bias = bool(np.any(np.asarray(inputs["bqkv"], dtype=np.float32)))
    nc = _get_program(with_bias=with_bias, debug=debug)
    in_maps = shard_inputs(**inputs)
    res = run_bass_kernel_spmd(
        nc, in_maps, core_ids=list(range(NCORES)), trace=trace, tmpdir=tmpdir,
    )
    return res


def combine(results, bout):
    bout = np.asarray(bout, dtype=np.float32)
    out = np.empty((B, T, D), dtype=np.float32)
    for b in range(B):
        acc = results[4 * b]["OUT"].astype(np.float32)
        for c in range(4 * b + 1, 4 * b + 4):
            acc = acc + results[c]["OUT"].astype(np.float32)
        out[b] = acc + bout[None, :]
    return out


def kernel(x, Wqkv, bqkv, Wout, bout):
    res = run_cores(dict(x=x, Wqkv=Wqkv, bqkv=bqkv, Wout=Wout, bout=bout))
    return combine(res.results, bout)


# revision 40
# speedup vs baseline: 1.1459x; 1.1459x over previous
"""Causal self-attention with rotary embeddings (B=2, T=2048, D=1024, H=16,
d_k=64) on 8 Trainium2 NeuronCores.

Sharding: core c handles batch b = c//4 and 4 heads (c%4)*4..+4 — data
parallel on B, tensor parallel on heads.  Each core computes its heads'
qkv projection, RoPE, causal attention, and a partial output projection
over its 256 attention channels; the host sums the 4 partials per batch.

Layout/perf notes:
  * everything is bf16 except PSUM accumulation, softmax scores (f32 in
    PSUM) and the reciprocal: x/Wqkv/Wout are cast host-side, q/k post
    rope, exp outputs, v, attn and the output partials are bf16
    (validated ~4e-3 rel err vs the 2e-2 gate).
  * q/k channels are de-interleaved host-side (RoPE pair -> half-split
    form) and packed 2 heads per 128-partition tile; the two heads'
    64-contraction score matmuls co-execute on separate PE row tiles.
  * RoPE swap (+/- sign) is a 128x128 permutation matmul on TensorE;
    cos/sin tables are natural scale, the 1/sqrt(d_k) folds into the
    exp's scale operand on the activation engine.
  * causal structure at 128-column granularity: score/av matmuls and
    exp only cover columns >= the key chunk's diagonal.
  * softmax denominator rides as a ones-column of v through the av
    matmul; reciprocal reads the PSUM row directly, gpsimd broadcasts,
    vector applies it during attnT eviction.
  * zero bias (the spec fills bqkv with zeros) skips the bias rank-1
    matmuls entirely; a with_bias program variant keeps generality.
  * the exp activation-table set is preloaded by a dummy ACTIVATE at
    t=0 so the 2.7us ACT_TABLE_LOAD overlaps the DMA ramp instead of
    stalling the first real exp.
"""

import sys

sys.path.insert(0, "/opt/trn_rl_repo")

import numpy as np
import ml_dtypes

import concourse.bacc as bacc
import concourse.tile as tile
from concourse import mybir
from concourse.bass_utils import run_bass_kernel_spmd

F32 = mybir.dt.float32
F32R = mybir.dt.float32r
BF16 = mybir.dt.bfloat16

B, T, D = 2, 2048, 1024
NH, DK = 16, 64
THETA = 10000.0
NCORES = 8
HEADS_PER_CORE = 4

TC512 = T // 512        # 4   i-chunks of 512
TC128 = T // 128        # 16  t/j-chunks of 128
KC = D // 128           # 8   d_model contraction chunks

EXP = mybir.ActivationFunctionType.Exp
COPYF = mybir.ActivationFunctionType.Copy


def build_program(with_bias=False, debug=False):
    nc = bacc.Bacc("TRN2", target_bir_lowering=False, debug=False)

    XT = nc.dram_tensor("XT", [D + 1, T], BF16, kind="ExternalInput").ap()
    WQK = nc.dram_tensor("WQK", [D + 1, 512], BF16, kind="ExternalInput").ap()
    WV = nc.dram_tensor("WV", [D + 1, 256], BF16, kind="ExternalInput").ap()
    WOUT = nc.dram_tensor("WOUT", [256, D], BF16, kind="ExternalInput").ap()
    PT = nc.dram_tensor("PT", [128, 256], BF16, kind="ExternalInput").ap()
    CSQ = nc.dram_tensor("CSQ", [128, 2 * T], BF16, kind="ExternalInput").ap()
    ONES64 = nc.dram_tensor("ONES64", [1, 64], F32R, kind="ExternalInput").ap()
    OUT = nc.dram_tensor("OUT", [T, D], BF16, kind="ExternalOutput").ap()
    if debug:
        DBG_QKT = nc.dram_tensor("DBG_QKT", [128, 4 * T], BF16, kind="ExternalOutput").ap()
        DBG_V = nc.dram_tensor("DBG_V", [128, TC128 * 260], BF16, kind="ExternalOutput").ap()
        DBG_ATT = nc.dram_tensor("DBG_ATT", [128, 2 * T], BF16, kind="ExternalOutput").ap()

    with tile.TileContext(nc) as tc:
        with (
            tc.tile_pool(name="persist", bufs=1) as persist,
            tc.tile_pool(name="p1w", bufs=1) as p1w,
            tc.tile_pool(name="p1t", bufs=3) as p1t,
            tc.tile_pool(name="p2e", bufs=4) as p2e,
            tc.tile_pool(name="p2bc", bufs=2) as p2bc,
            tc.tile_pool(name="p2r", bufs=2) as p2r,
            tc.tile_pool(name="pj", bufs=2, space="PSUM") as pj,
            tc.tile_pool(name="sps", bufs=2, space="PSUM") as sps,
            tc.tile_pool(name="avps", bufs=2, space="PSUM") as avps,
        ):
            # ---- persistent tiles --------------------------------------
            qkT = persist.tile([128, 4 * T], BF16, tag="qkT")       # Qp0 Kp0 Qp1 Kp1
            v_sb = persist.tile([128, TC128 * 260], BF16, tag="v_sb")  # [jc, head, 64+1]
            attnT = persist.tile([128, 2 * T], BF16, tag="attnT")   # c-chunks x t
            wout_sb = persist.tile([128, 2 * D], BF16, tag="wout_sb")
            ones_sb = persist.tile([1, 64], F32R, tag="ones_sb")
            warm_sb = persist.tile([1, 8], F32, tag="warm_sb")

            x_sb = p1w.tile([128, KC * T], BF16, tag="x_sb")
            wqk_sb = p1w.tile([128, KC * 512], BF16, tag="wqk_sb")
            wv_sb = p1w.tile([128, KC * 256], BF16, tag="wv_sb")
            pt_sb = p1w.tile([128, 256], BF16, tag="pt_sb")
            psw_sb = pt_sb[:, 0:128]
            tri_sb = pt_sb[:, 128:256]
            csq_sb = p1w.tile([128, 2 * T], BF16, tag="csq_sb")
            cq_sb = csq_sb[:, 0:T]
            sq_sb = csq_sb[:, T:2 * T]
            if with_bias:
                xlast = p1w.tile([1, T], BF16, tag="xlast")
                wqk_last = p1w.tile([1, 512], BF16, tag="wqk_last")
                wv_last = p1w.tile([1, 256], BF16, tag="wv_last")

            xt_src = XT[0:D, :].rearrange("(k p) t -> p k t", p=128)
            x_dst = x_sb[:].rearrange("p (k t) -> p k t", k=KC)

            # x block n: 2 batched triggers (k 0..3 / 4..7) on 2 queues
            def load_x_block(n, engines=(nc.sync, nc.gpsimd)):
                nsl = slice(n * 512, (n + 1) * 512)
                for half, eng in enumerate(engines):
                    ks = slice(half * 4, half * 4 + 4)
                    eng.dma_start(x_dst[:, ks, nsl], xt_src[:, ks, nsl])

            # warm the exp table on ScalarE while DMA ramps: the first
            # ACTIVATE of a program pays a ~2.7us ACT_TABLE_LOAD; a dummy
            # exp at t=0 moves that load under the DMA ramp.
            nc.vector.memset(warm_sb[:], 0.0)
            nc.scalar.activation(warm_sb[:], warm_sb[:], EXP, scale=0.125)

            # ---- preamble loads: k-chunk granular, pipelined to match the
            # PE's consumption order (one (wqk_k, x_k) pair per ~0.85us),
            # interleaved across the 3 DMA-trigger queues (SP/Act/gpsimd).
            wqk_dst = wqk_sb[:].rearrange("p (k c) -> p k c", k=KC)
            wqk_src = WQK[0:D, :].rearrange("(k p) c -> p k c", p=128)
            wv_dst = wv_sb[:].rearrange("p (k c) -> p k c", k=KC)
            wv_src = WV[0:D, :].rearrange("(k p) c -> p k c", p=128)
            for k in range(4):
                nc.sync.dma_start(wqk_dst[:, k], wqk_src[:, k])
                nc.gpsimd.dma_start(x_dst[:, k, 0:512], xt_src[:, k, 0:512])
            nc.scalar.dma_start(pt_sb[:], PT[:])
            nc.scalar.dma_start(csq_sb[:], CSQ[:])
            for k in range(4, 8):
                nc.sync.dma_start(x_dst[:, k, 0:512], xt_src[:, k, 0:512])
                nc.gpsimd.dma_start(wqk_dst[:, k], wqk_src[:, k])
            nc.scalar.dma_start(wv_dst[:, 0:4], wv_src[:, 0:4])
            nc.gpsimd.dma_start(wv_dst[:, 4:8], wv_src[:, 4:8])
            nc.gpsimd.dma_start(ones_sb[:], ONES64[:])
            if with_bias:
                nc.gpsimd.dma_start(wqk_last[:], WQK[D:D + 1, :])
                nc.gpsimd.dma_start(xlast[:], XT[D:D + 1, :])
                nc.gpsimd.dma_start(wv_last[:], WV[D:D + 1, :])

            # ones columns of v_aug: one strided memset
            v4 = v_sb[:].rearrange("p (jc h e) -> p jc h e", jc=TC128, h=4)
            nc.vector.memset(v4[:, :, :, 64:65], 1.0)

            # ---------------- building blocks ---------------------------
            def qk_proj_chunk(m, n):
                """project q/k m-chunk (128 channels) for t-chunk n (512), apply rope."""
                nsl = slice(n * 512, (n + 1) * 512)
                ps = pj.tile([128, 512], F32, tag="pj", name=f"psqk_{m}_{n}")
                for k in range(KC):
                    nc.tensor.matmul(
                        ps[:],
                        wqk_sb[:, k * 512 + m * 128:k * 512 + (m + 1) * 128],
                        x_sb[:, k * T + n * 512:k * T + (n + 1) * 512],
                        start=(k == 0), stop=(not with_bias and k == KC - 1),
                    )
                if with_bias:
                    nc.tensor.matmul(
                        ps[:], wqk_last[:, m * 128:(m + 1) * 128], xlast[:, nsl],
                        start=False, stop=True,
                    )
                tmp_s = p1t.tile([128, 512], BF16, tag="tmp_s", name=f"tmps_{m}_{n}")
                tmp_c = p1t.tile([128, 512], BF16, tag="tmp_c", name=f"tmpc_{m}_{n}")
                nc.vector.tensor_mul(tmp_s[:], ps[:], sq_sb[:, nsl])
                nc.vector.tensor_mul(tmp_c[:], ps[:], cq_sb[:, nsl])
                sw = pj.tile([128, 512], F32, tag="pj", name=f"sw_{m}_{n}")
                nc.tensor.matmul(sw[:], psw_sb, tmp_s[:], start=True, stop=True)
                nc.vector.tensor_add(qkT[:, m * T + n * 512:m * T + (n + 1) * 512], sw[:], tmp_c[:])

            def v_proj_chunk(tcc):
                tsl = slice(tcc * 128, (tcc + 1) * 128)
                psv = pj.tile([128, 256], F32, tag="pj", name=f"psv_{tcc}")
                for k in range(KC):
                    nc.tensor.matmul(
                        psv[:],
                        x_sb[:, k * T + tcc * 128:k * T + (tcc + 1) * 128],
                        wv_sb[:, k * 256:(k + 1) * 256],
                        start=(k == 0), stop=(not with_bias and k == KC - 1),
                    )
                if with_bias:
                    nc.tensor.matmul(psv[:], xlast[:, tsl], wv_last[:], start=False, stop=True)
                vdst = v_sb[:, tcc * 260:(tcc + 1) * 260].rearrange(
                    "p (h e) -> p h e", h=4)[:, :, 0:64]
                vsrc = psv[:].rearrange("p (h e) -> p h e", e=64)
                nc.vector.tensor_copy(vdst, vsrc)

            def attn_ic(p, ic, fillers=(), last=False):
                """attention for head-pair p, query chunk ic (512 queries).
                fillers: callables run one per jc iteration (PE density)."""
                fillers = list(fillers)
                qof = (2 * p) * T
                kof = (2 * p + 1) * T
                njc = 4 * ic + 4
                av = [avps.tile([65, 512], F32, tag="av", name=f"av_{p}_{ic}_{i}") for i in range(2)]
                for jc in range(njc):
                    rel = jc - 4 * ic
                    ls = 0 if rel < 0 else rel * 128
                    e_pair = p2e.tile([128, 1024], BF16, tag="e_t", name=f"e_{p}_{ic}_{jc}")
                    s_pair = sps.tile([128, 1024], F32, tag="s_ps", name=f"s_{p}_{ic}_{jc}")
                    for hh in range(2):
                        pof = hh * 64
                        nc.tensor.matmul(
                            s_pair[:, hh * 512 + ls:(hh + 1) * 512],
                            qkT[pof:pof + 64, kof + jc * 128:kof + (jc + 1) * 128],
                            qkT[pof:pof + 64, qof + ic * 512 + ls:qof + (ic + 1) * 512],
                            start=True, stop=True,
                        )
                    sv = s_pair[:].rearrange("p (h w) -> p h w", h=2)
                    ev = e_pair[:].rearrange("p (h w) -> p h w", h=2)
                    nc.scalar.activation(ev[:, :, ls:512], sv[:, :, ls:512], EXP, scale=0.125)
                    if rel >= 0:
                        tsl_ = slice(rel * 128, (rel + 1) * 128)
                        nc.vector.tensor_mul(ev[:, :, tsl_], ev[:, :, tsl_],
                                             tri_sb.unsqueeze(1).broadcast_to([128, 2, 128]))
                    for hh in range(2):
                        nc.tensor.matmul(
                            av[hh][:, ls:512],
                            v_sb[:, jc * 260 + (2 * p + hh) * 65:jc * 260 + (2 * p + hh) * 65 + 65],
                            e_pair[:, hh * 512 + ls:(hh + 1) * 512],
                            start=(jc == 0), stop=(jc == njc - 1),
                            skip_group_check=True,
                        )
                    if fillers and (jc % max(1, njc // len(fillers)) == 0 or jc == njc - 1):
                        while fillers and len(fillers) > (njc - 1 - jc):
                            fillers.pop(0)()
                for hh in range(2):
                    head = 2 * p + hh
                    cof = (head // 2) * T
                    pof = (head % 2) * 64
                    den = p2r.tile([1, 512], F32, tag="den", name=f"den_{p}_{ic}_{hh}")
                    nc.vector.tensor_copy(den[:], av[hh][64:65, :])
                    rec = p2r.tile([1, 512], F32, tag="rec", name=f"rec_{p}_{ic}_{hh}")
                    nc.vector.reciprocal_approx_fast(rec[:], den[:])
                    dst = attnT[pof:pof + 64, cof + ic * 512:cof + (ic + 1) * 512]
                    bc_sb = p2bc.tile([64, 512], F32, tag="bc_sb", name=f"bc_{p}_{ic}_{hh}")
                    nc.gpsimd.partition_broadcast(bc_sb[:], rec[:], channels=64)
                    nc.vector.tensor_mul(dst, av[hh][0:64, :], bc_sb[:])

            def out_proj_chunk(tcc):
                tsl = slice(tcc * 128, (tcc + 1) * 128)
                for oc in range(2):
                    po = pj.tile([128, 512], F32, tag="pj", name=f"po_{tcc}_{oc}")
                    for cc in range(2):
                        nc.tensor.matmul(
                            po[:],
                            attnT[:, cc * T + tcc * 128:cc * T + (tcc + 1) * 128],
                            wout_sb[:, cc * D + oc * 512:cc * D + (oc + 1) * 512],
                            start=(cc == 0), stop=(cc == 1),
                        )
                    osl = slice(oc * 512, (oc + 1) * 512)
                    po_sb = p1t.tile([128, 512], BF16, tag="po_sb", name=f"po_sb_{tcc}_{oc}")
                    if oc == 0:
                        nc.vector.tensor_copy(po_sb[:], po[:])
                    else:
                        nc.scalar.copy(po_sb[:], po[:])
                    nc.sync.dma_start(OUT[tsl, osl], po_sb[:])

            # ---------------- schedule: n-major waves -------------------
            load_x_block(1, engines=(nc.scalar, nc.sync))
            for m in range(4):
                qk_proj_chunk(m, 0)
            for tcc in range(4):
                v_proj_chunk(tcc)
            for n in range(TC512):
                fill0, fill1 = [], []
                if n < 3:
                    nx = n + 1
                    if nx + 1 < TC512:
                        fill0 += [lambda b=nx + 1: load_x_block(b)]
                    fill0 += [(lambda m=m: qk_proj_chunk(m, nx)) for m in range(4)]
                    fill0 += [(lambda t=t: v_proj_chunk(t)) for t in range(4 * nx, 4 * nx + 4)]
                if n == 0:
                    def load_wout():
                        nc.scalar.dma_start(wout_sb[:, 0:D], WOUT[0:128, :])
                        nc.gpsimd.dma_start(wout_sb[:, D:2 * D], WOUT[128:256, :])
                    fill0 += [load_wout]
                op_sched = {1: range(0, 4), 2: range(4, 8), 3: range(8, 12)}
                if n in op_sched:
                    fill1 += [(lambda t=t: out_proj_chunk(t)) for t in op_sched[n]]
                half = len(fill0) // 2
                is_last = (n == TC512 - 1)
                attn_ic(0, n, fill0[:half] + fill1[:2], last=is_last)
                attn_ic(1, n, fill0[half:] + fill1[2:], last=is_last)
            for tcc in range(12, 16):
                out_proj_chunk(tcc)

            if debug:
                nc.sync.dma_start(DBG_QKT[:], qkT[:])
                nc.sync.dma_start(DBG_V[:], v_sb[:])
                nc.sync.dma_start(DBG_ATT[:], attnT[:])

    nc.compile()
    return nc


_DEINT = list(range(0, DK, 2)) + list(range(1, DK, 2))


def _rope_tables():
    j = np.arange(DK // 2, dtype=np.float64)
    inv_freq = THETA ** (-2.0 * j / DK)
    t = np.arange(T, dtype=np.float64)
    ang = t[None, :] * inv_freq[:, None]          # [32, T]
    ang = np.tile(ang, (4, 1))                    # [128, T]
    return np.cos(ang), np.sin(ang)


def _psw():
    M = np.zeros((128, 128), dtype=np.float32)
    for p in range(128):
        pm = p % 64
        if pm < 32:
            M[p, p + 32] = -1.0
        else:
            M[p, p - 32] = 1.0
    return np.ascontiguousarray(M.T)


def shard_inputs(x, Wqkv, bqkv, Wout, bout):
    bf = ml_dtypes.bfloat16
    x = np.asarray(x, dtype=np.float32)
    Wqkv = np.asarray(Wqkv, dtype=np.float32)
    bqkv = np.asarray(bqkv, dtype=np.float32)
    Wout = np.asarray(Wout, dtype=np.float32)

    cos_t, sin_t = _rope_tables()
    csq = np.ascontiguousarray(
        np.concatenate([cos_t, sin_t], axis=1)).astype(bf)   # [128, 2T]
    psw = _psw()
    tri = np.triu(np.ones((128, 128), dtype=np.float32))
    pt = np.ascontiguousarray(np.concatenate([psw, tri], axis=1)).astype(bf)
    ones64 = np.ones((1, 64), dtype=np.float32)

    Wfull = np.concatenate([Wqkv, bqkv[:, None]], axis=1)  # [3072, 1025]

    xt = {}
    for b in range(B):
        xt[b] = np.ascontiguousarray(
            np.concatenate([x[b].T, np.ones((1, T), np.float32)], axis=0)
        ).astype(bf)

    in_maps = []
    for c in range(NCORES):
        b = c // 4
        heads = [4 * (c % 4) + i for i in range(HEADS_PER_CORE)]
        # chunk order: [Qp0 | Kp0 | Qp1 | Kp1], each 128 rows (2 heads x 64)
        qk_rows = []
        for p in range(2):
            qrows, krows = [], []
            for h in (2 * p, 2 * p + 1):
                H = heads[h]
                qrows += [H * 192 + j for j in _DEINT]
                krows += [H * 192 + 64 + j for j in _DEINT]
            qk_rows += qrows + krows
        v_rows = []
        for h in range(4):
            H = heads[h]
            v_rows += [H * 192 + 128 + j for j in range(DK)]
        vch_out = []
        for h in range(4):
            H = heads[h]
            vch_out += [H * 64 + j for j in range(DK)]

        in_maps.append({
            "XT": xt[b],
            "WQK": np.ascontiguousarray(Wfull[qk_rows].T).astype(bf),
            "WV": np.ascontiguousarray(Wfull[v_rows].T).astype(bf),
            "WOUT": np.ascontiguousarray(Wout[:, vch_out].T).astype(bf),
            "PT": pt,
            "CSQ": csq,
            "ONES64": ones64,
        })
    return in_maps


_CACHED = {}


def _get_program(with_bias=False, debug=False):
    key = (bool(with_bias), bool(debug))
    if key not in _CACHED:
        _CACHED[key] = build_program(with_bias=with_bias, debug=debug)
    return _CACHED[key]


def run_cores(inputs, debug=False, trace=False, tmpdir=None):
    with_bias = bool(np.any(np.asarray(inputs["bqkv"], dtype=np.float32)))
    nc = _get_program(with_bias=with_bias, debug=debug)
    in_maps = shard_inputs(**inputs)
    res = run_bass_kernel_spmd(
        nc, in_maps, core_ids=list(range(NCORES)), trace=trace, tmpdir=tmpdir,
    )
    return res


def combine(results, bout):
    bout = np.asarray(bout, dtype=np.float32)
    out = np.empty((B, T, D), dtype=np.float32)
    for b in range(B):
        acc = results[4 * b]["OUT"].astype(np.float32)
        for c in range(4 * b + 1, 4 * b + 4):
            acc = acc + results[c]["OUT"].astype(np.float32)
        out[b] = acc + bout[None, :]
    return out


def kernel(x, Wqkv, bqkv, Wout, bout):
    res = run_cores(dict(x=x, Wqkv=Wqkv, bqkv=bqkv, Wout=Wout, bout=bout))
    return combine(res.results, bout)
